# revision 15
# baseline (speedup 1.0000x reference)
"""Trainium2 Bass kernel for BertForSpanAspectExtraction span scoring.

Computes, for x = sequence_output [B=4, L=256, H=768]:
  start_logits = x @ w_start + b_start                      [B, L]
  end_logits   = x @ w_end   + b_end                        [B, L]
  span_sig     = sigmoid(relu(s_i + e_j + b1) @ W2 + b2)    [B, L, L]
with s = x @ W1s, e = x @ W1e  (H2 = 384).

Sharding: 8 cores = (b, i-half).  Each core receives a column-rotated x[b].T
so its own 128 start-rows are always columns 0..127 (uniform SPMD program);
the host un-rotates the span columns after gathering.

Per core:
  - Inputs arrive host-pre-tiled partition-major so each tensor loads with one
    DMA of 128 large descriptors, spread over the SP/ACT HWDGE rings + SWDGE.
  - PE: bf16 projections e_T [384,256], s_T [384,128] (k on partitions),
    fp32 start/end logit rows.
  - Grid: per (i, kblock) one fused h = relu(e_T + s_col) op ([128,256] bf16,
    per-partition-scalar broadcast), split across ScalarE / VectorE / GpSimd;
    pairs of i's share one [128,512] rhs so each K=128 W2-reduction matmul
    streams N=512 into a full PSUM bank at col-group partition 32*(i%4).
  - Sigmoid evacuates two banks [128,1024] per ScalarE op (16 span rows).
"""

import sys

if "/opt/trn_rl_repo" not in sys.path:
    sys.path.insert(0, "/opt/trn_rl_repo")

import ml_dtypes
import numpy as np

B, L, H = 4, 256, 768
H2 = 384
NCORES = 8
LH = 128  # start-rows per core
NH = H // 128  # 6 contraction blocks
NK = H2 // 128  # 3 k blocks

# grid-op engine split (of 384 ops): ScalarE share, rest VectorE
ACT_OPS = 86
H_BUFS = 12

_built = {}


def _build():
    import concourse.bacc as bacc
    import concourse.mybir as mybir
    import concourse.tile as tile

    f32 = mybir.dt.float32
    bf16 = mybir.dt.bfloat16
    AF = mybir.ActivationFunctionType
    OP = mybir.AluOpType

    nc = bacc.Bacc("TRN2", debug=False, target_bir_lowering=False, enable_asserts=False)

    # host-pre-tiled, partition-major inputs (one DMA each, 128 big descriptors)
    xT = nc.dram_tensor("xT", [128, NH * L], f32, kind="ExternalInput").ap()
    W1e = nc.dram_tensor("W1e", [128, NH * H2], bf16, kind="ExternalInput").ap()
    W1s = nc.dram_tensor("W1s", [128, NH * H2], bf16, kind="ExternalInput").ap()
    W2r = nc.dram_tensor("W2r", [128, NK * 32], bf16, kind="ExternalInput").ap()
    # smalls cols: b1[kb]@kb (3) | w_start[hb]@3+hb (6) | w_end[hb]@9+hb (6) | b2@15
    sml = nc.dram_tensor("sml", [128, 16], f32, kind="ExternalInput").ap()

    span = nc.dram_tensor("span", [LH, L], f32, kind="ExternalOutput").ap()
    slog = nc.dram_tensor("slog", [LH], f32, kind="ExternalOutput").ap()
    elog = nc.dram_tensor("elog", [LH], f32, kind="ExternalOutput").ap()

    with tile.TileContext(nc) as tc:
        with (
            tc.tile_pool(name="persist", bufs=1) as pp,
            tc.tile_pool(name="hpool", bufs=H_BUFS) as hp,
            tc.tile_pool(name="psum", bufs=2, space="PSUM") as pq,
            tc.tile_pool(name="stage", bufs=2) as sp,
        ):
            # ---------------- input loads (3 rings in parallel) ----------------
            xta = pp.tile([128, NH * L], f32, tag="xta", name="xta")
            nc.sync.dma_start(xta[:], xT[:])
            w1e = pp.tile([128, NH * H2], bf16, tag="w1e", name="w1e")
            nc.scalar.dma_start(w1e[:], W1e[:])
            w1s = pp.tile([128, NH * H2], bf16, tag="w1s", name="w1s")
            nc.gpsimd.dma_start(w1s[:], W1s[:])
            w2a = pp.tile([128, NK * 32], bf16, tag="w2a", name="w2a")
            nc.gpsimd.dma_start(w2a[:], W2r[:])
            sm = pp.tile([128, 16], f32, tag="sm", name="sm")
            nc.gpsimd.dma_start(sm[:], sml[:])

            def xt(hb):
                return xta[:, hb * L : (hb + 1) * L]

            def w1(t, hb, kb):
                return t[:, hb * H2 + kb * 128 : hb * H2 + (kb + 1) * 128]

            # bf16 copies of xT for the (all-bf16) projection matmuls
            xtb = [
                pp.tile([128, L], bf16, tag=f"xtb{hb}", name=f"xtb{hb}")
                for hb in range(NH)
            ]
            for hb in range(NH):
                nc.vector.tensor_copy(xtb[hb][:], xt(hb))

            # ---------------- projections ----------------
            eT = [
                pp.tile([128, L], bf16, tag=f"eT{kb}", name=f"eT{kb}")
                for kb in range(NK)
            ]
            sT = [
                pp.tile([128, LH], f32, tag=f"sT{kb}", name=f"sT{kb}")
                for kb in range(NK)
            ]
            for kb in range(NK):
                pe = pq.tile([128, 2048], f32, tag="psum")
                for hb in range(NH):
                    nc.tensor.matmul(
                        pe[:, 0:L],
                        w1(w1e, hb, kb),
                        xtb[hb][:],
                        start=(hb == 0),
                        stop=(hb == NH - 1),
                    )
                nc.scalar.activation(eT[kb][:], pe[:, 0:L], AF.Identity, bias=0.0)
            for kb in range(NK):
                ps = pq.tile([128, 2048], f32, tag="psum")
                for hb in range(NH):
                    nc.tensor.matmul(
                        ps[:, 0:LH],
                        w1(w1s, hb, kb),
                        xtb[hb][:, 0:LH],
                        start=(hb == 0),
                        stop=(hb == NH - 1),
                    )
                # fold b1 into the mandatory PSUM->SBUF evacuation
                nc.scalar.activation(
                    sT[kb][:], ps[:, 0:LH], AF.Identity, bias=sm[:, kb : kb + 1]
                )

            # ---------------- span grid ----------------
            # local row i = 32*batch + m*4 + cg at PSUM partition 32*cg, free
            # m*256; i-pairs (m=2mp, 2mp+1) share one [128,512] bf16 rhs and
            # one full-bank accumulation group per (cg, bank=mp)
            opct = 0
            n_grid_ops = LH * NK
            dstv = span.rearrange("(bt m cg) j -> bt m cg j", bt=LH // 32, m=8, cg=4)
            for batch in range(LH // 32):
                pt = pq.tile([128, 2048], f32, tag="psum")
                st = sp.tile([128, 2048], f32, tag="stage")
                for mp in range(4):
                    for kb in range(NK):
                        for cg in range(4):
                            h2 = hp.tile([128, 512], bf16, tag="h")
                            for half in range(2):
                                i = batch * 32 + (2 * mp + half) * 4 + cg
                                dst = h2[:, half * 256 : (half + 1) * 256]
                                t = opct
                                opct += 1
                                if (t * ACT_OPS) % n_grid_ops < ACT_OPS:
                                    nc.scalar.activation(
                                        dst,
                                        eT[kb][:],
                                        AF.Relu,
                                        bias=sT[kb][:, i : i + 1],
                                    )
                                else:
                                    nc.vector.tensor_scalar(
                                        dst,
                                        eT[kb][:],
                                        sT[kb][:, i : i + 1],
                                        0.0,
                                        op0=OP.add,
                                        op1=OP.max,
                                    )
                            nc.tensor.matmul(
                                pt[32 * cg : 32 * cg + 32, mp * 512 : (mp + 1) * 512],
                                w2a[:, kb * 32 : (kb + 1) * 32],
                                h2[:],
                                start=(kb == 0),
                                stop=(kb == NK - 1),
                                tile_position=(0, 32 * cg),
                                # sim's group-check mis-maps partition-offset
                                # outputs; functional pending-zero model is fine
                                skip_group_check=True,
                            )
                for half in range(2):
                    nc.scalar.activation(
                        st[:, half * 1024 : (half + 1) * 1024],
                        pt[:, half * 1024 : (half + 1) * 1024],
                        AF.Sigmoid,
                        bias=sm[:, 15:16],
                    )
                for cg in range(4):
                    src = st[32 * cg : 32 * cg + 1, :].rearrange(
                        "p (m j) -> p m j", m=8
                    )
                    nc.sync.dma_start(dstv[batch, :, cg, :], src)

            # ---------------- start/end logits (fp32, exact) ----------------
            for col0, outdram in ((3, slog), (9, elog)):
                pl = pq.tile([1, 128], f32, tag="psum")
                for hb in range(NH):
                    nc.tensor.matmul(
                        pl[:],
                        sm[:, col0 + hb : col0 + hb + 1],
                        xt(hb)[:, 0:LH],
                        start=(hb == 0),
                        stop=(hb == NH - 1),
                    )
                lt = sp.tile([1, 128], f32, tag="lt")
                nc.vector.tensor_copy(lt[:], pl[:])
                nc.sync.dma_start(outdram[:], lt[:])


    nc.compile()
    return nc


def _get_nc():
    if "nc" not in _built:
        _built["nc"] = _build()
    return _built["nc"]


def _make_in_maps(
    sequence_output, w_start, b_start, w_end, b_end, W1s, W1e, b1, W2, b2
):
    x = np.asarray(sequence_output, dtype=np.float32)

    def pack_w1(w):  # [H, H2] -> [128, NH*H2] partition-major bf16
        w = np.asarray(w, np.float32).reshape(NH, 128, H2).transpose(1, 0, 2)
        return np.ascontiguousarray(w.reshape(128, NH * H2)).astype(ml_dtypes.bfloat16)

    sml = np.zeros((128, 16), np.float32)
    sml[:, 0:3] = np.asarray(b1, np.float32).reshape(3, 128).T
    sml[:, 3:9] = np.asarray(w_start, np.float32).reshape(6, 128).T
    sml[:, 9:15] = np.asarray(w_end, np.float32).reshape(6, 128).T
    sml[:, 15] = float(np.asarray(b2))
    w2r = np.repeat(
        np.asarray(W2, np.float32).reshape(NK, 128, 1).transpose(1, 0, 2), 32, axis=2
    )  # [128, NK, 32]
    shared = {
        "W1e": pack_w1(W1e),
        "W1s": pack_w1(W1s),
        "W2r": np.ascontiguousarray(w2r.reshape(128, NK * 32)).astype(
            ml_dtypes.bfloat16
        ),
        "sml": sml,
    }
    in_maps = []
    for core in range(NCORES):
        b, ih = core // 2, core % 2
        xTr = np.roll(x[b].T, -ih * LH, axis=1)  # [H, L], own rows at cols 0..127
        xp = xTr.reshape(NH, 128, L).transpose(1, 0, 2)  # [128, NH, L]
        m = dict(shared)
        m["xT"] = np.ascontiguousarray(xp.reshape(128, NH * L))
        in_maps.append(m)
    return in_maps


def kernel(
    sequence_output, w_start, b_start, w_end, b_end, W1s, W1e, b1, W2, b2
):
    from concourse.bass_utils import run_bass_kernel_spmd

    nc = _get_nc()
    in_maps = _make_in_maps(
        sequence_output, w_start, b_start, w_end, b_end, W1s, W1e, b1, W2, b2
    )
    res = run_bass_kernel_spmd(nc, in_maps, core_ids=list(range(NCORES)))
    span = np.empty((B, L, L), np.float32)
    sl = np.empty((B, L), np.float32)
    el = np.empty((B, L), np.float32)
    bs = float(np.asarray(b_start))
    be = float(np.asarray(b_end))
    for core in range(NCORES):
        b, ih = core // 2, core % 2
        r = res.results[core]
        span[b, ih * LH : (ih + 1) * LH, :] = np.roll(r["span"], ih * LH, axis=1)
        sl[b, ih * LH : (ih + 1) * LH] = r["slog"] + bs
        el[b, ih * LH : (ih + 1) * LH] = r["elog"] + be
    return (sl, el, span)


# revision 16
# speedup vs baseline: 1.1775x; 1.1775x over previous
"""Trainium2 Bass kernel for BertForSpanAspectExtraction span scoring.

Computes, for x = sequence_output [B=4, L=256, H=768]:
  start_logits = x @ w_start + b_start                      [B, L]
  end_logits   = x @ w_end   + b_end                        [B, L]
  span_sig     = sigmoid(relu(s_i + e_j + b1) @ W2 + b2)    [B, L, L]
with s = x @ W1s, e = x @ W1e  (H2 = 384).

Sharding: 8 cores = (b, i-half).  Each core receives a column-rotated x[b].T
so its own 128 start-rows are always columns 0..127 (uniform SPMD program);
the host un-rotates the span columns after gathering.

Per core:
  - Inputs arrive host-pre-tiled partition-major so each tensor loads with one
    DMA of 128 large descriptors, spread over the SP/ACT HWDGE rings + SWDGE.
  - PE: bf16 projections e_T [384,256], s_T [384,128] (k on partitions),
    fp32 start/end logit rows.
  - Grid: per (i, kblock) one fused h = relu(e_T + s_col) op ([128,256] bf16,
    per-partition-scalar broadcast), split across ScalarE / VectorE / GpSimd;
    pairs of i's share one [128,512] rhs so each K=128 W2-reduction matmul
    streams N=512 into a full PSUM bank at col-group partition 32*(i%4).
  - Sigmoid evacuates two banks [128,1024] per ScalarE op (16 span rows).
"""

import sys

if "/opt/trn_rl_repo" not in sys.path:
    sys.path.insert(0, "/opt/trn_rl_repo")

import ml_dtypes
import numpy as np

B, L, H = 4, 256, 768
H2 = 384
NCORES = 8
LH = 128  # start-rows per core
NH = H // 128  # 6 contraction blocks
NK = H2 // 128  # 3 k blocks

# grid-op engine split (of 384 ops): ScalarE share, rest VectorE
ACT_OPS = 86
H_BUFS = 12

_built = {}


def _build():
    import concourse.bacc as bacc
    import concourse.mybir as mybir
    import concourse.tile as tile

    f32 = mybir.dt.float32
    bf16 = mybir.dt.bfloat16
    AF = mybir.ActivationFunctionType
    OP = mybir.AluOpType

    nc = bacc.Bacc("TRN2", debug=False, target_bir_lowering=False)

    # host-pre-tiled, partition-major inputs (one DMA each, 128 big descriptors)
    xT = nc.dram_tensor("xT", [128, NH * L], f32, kind="ExternalInput").ap()
    W1e = nc.dram_tensor("W1e", [128, NH * H2], bf16, kind="ExternalInput").ap()
    W1s = nc.dram_tensor("W1s", [128, NH * H2], bf16, kind="ExternalInput").ap()
    W2r = nc.dram_tensor("W2r", [128, NK * 32], bf16, kind="ExternalInput").ap()
    # smalls cols: b1[kb]@kb (3) | w_start[hb]@3+hb (6) | w_end[hb]@9+hb (6) | b2@15
    sml = nc.dram_tensor("sml", [128, 16], f32, kind="ExternalInput").ap()

    span = nc.dram_tensor("span", [LH, L], f32, kind="ExternalOutput").ap()
    slog = nc.dram_tensor("slog", [LH], f32, kind="ExternalOutput").ap()
    elog = nc.dram_tensor("elog", [LH], f32, kind="ExternalOutput").ap()

    with tile.TileContext(nc) as tc:
        with (
            tc.tile_pool(name="persist", bufs=1) as pp,
            tc.tile_pool(name="hpool", bufs=H_BUFS) as hp,
            tc.tile_pool(name="psum", bufs=2, space="PSUM") as pq,
            tc.tile_pool(name="stage", bufs=2) as sp,
        ):
            # ---------------- input loads (3 rings in parallel) ----------------
            xta = pp.tile([128, NH * L], f32, tag="xta", name="xta")
            nc.sync.dma_start(xta[:], xT[:])
            w1e = pp.tile([128, NH * H2], bf16, tag="w1e", name="w1e")
            nc.scalar.dma_start(w1e[:], W1e[:])
            w1s = pp.tile([128, NH * H2], bf16, tag="w1s", name="w1s")
            nc.gpsimd.dma_start(w1s[:], W1s[:])
            w2a = pp.tile([128, NK * 32], bf16, tag="w2a", name="w2a")
            nc.gpsimd.dma_start(w2a[:], W2r[:])
            sm = pp.tile([128, 16], f32, tag="sm", name="sm")
            nc.gpsimd.dma_start(sm[:], sml[:])

            def xt(hb):
                return xta[:, hb * L : (hb + 1) * L]

            def w1(t, hb, kb):
                return t[:, hb * H2 + kb * 128 : hb * H2 + (kb + 1) * 128]

            # bf16 copies of xT for the (all-bf16) projection matmuls
            xtb = [
                pp.tile([128, L], bf16, tag=f"xtb{hb}", name=f"xtb{hb}")
                for hb in range(NH)
            ]
            for hb in range(NH):
                nc.vector.tensor_copy(xtb[hb][:], xt(hb))

            # ---------------- projections ----------------
            eT = [
                pp.tile([128, L], bf16, tag=f"eT{kb}", name=f"eT{kb}")
                for kb in range(NK)
            ]
            sT = [
                pp.tile([128, LH], f32, tag=f"sT{kb}", name=f"sT{kb}")
                for kb in range(NK)
            ]
            for kb in range(NK):
                pe = pq.tile([128, 2048], f32, tag="psum")
                for hb in range(NH):
                    nc.tensor.matmul(
                        pe[:, 0:L],
                        w1(w1e, hb, kb),
                        xtb[hb][:],
                        start=(hb == 0),
                        stop=(hb == NH - 1),
                    )
                nc.scalar.activation(eT[kb][:], pe[:, 0:L], AF.Identity, bias=0.0)
            for kb in range(NK):
                ps = pq.tile([128, 2048], f32, tag="psum")
                for hb in range(NH):
                    nc.tensor.matmul(
                        ps[:, 0:LH],
                        w1(w1s, hb, kb),
                        xtb[hb][:, 0:LH],
                        start=(hb == 0),
                        stop=(hb == NH - 1),
                    )
                # fold b1 into the mandatory PSUM->SBUF evacuation
                nc.scalar.activation(
                    sT[kb][:], ps[:, 0:LH], AF.Identity, bias=sm[:, kb : kb + 1]
                )

            # ---------------- span grid ----------------
            # local row i = 32*batch + m*4 + cg at PSUM partition 32*cg, free
            # m*256; i-pairs (m=2mp, 2mp+1) share one [128,512] bf16 rhs and
            # one full-bank accumulation group per (cg, bank=mp)
            opct = 0
            n_grid_ops = LH * NK
            dstv = span.rearrange("(bt m cg) j -> bt m cg j", bt=LH // 32, m=8, cg=4)
            for batch in range(LH // 32):
                pt = pq.tile([128, 2048], f32, tag="psum")
                st = sp.tile([128, 2048], f32, tag="stage")
                for mp in range(4):
                    for kb in range(NK):
                        for cg in range(4):
                            h2 = hp.tile([128, 512], bf16, tag="h")
                            for half in range(2):
                                i = batch * 32 + (2 * mp + half) * 4 + cg
                                dst = h2[:, half * 256 : (half + 1) * 256]
                                t = opct
                                opct += 1
                                if (t * ACT_OPS) % n_grid_ops < ACT_OPS:
                                    nc.scalar.activation(
                                        dst,
                                        eT[kb][:],
                                        AF.Relu,
                                        bias=sT[kb][:, i : i + 1],
                                    )
                                else:
                                    nc.vector.tensor_scalar(
                                        dst,
                                        eT[kb][:],
                                        sT[kb][:, i : i + 1],
                                        0.0,
                                        op0=OP.add,
                                        op1=OP.max,
                                    )
                            nc.tensor.matmul(
                                pt[32 * cg : 32 * cg + 32, mp * 512 : (mp + 1) * 512],
                                w2a[:, kb * 32 : (kb + 1) * 32],
                                h2[:],
                                start=(kb == 0),
                                stop=(kb == NK - 1),
                                tile_position=(0, 32 * cg),
                                # sim's group-check mis-maps partition-offset
                                # outputs; functional pending-zero model is fine
                                skip_group_check=True,
                            )
                for half in range(2):
                    nc.scalar.activation(
                        st[:, half * 1024 : (half + 1) * 1024],
                        pt[:, half * 1024 : (half + 1) * 1024],
                        AF.Sigmoid,
                        bias=sm[:, 15:16],
                    )
                for cg in range(4):
                    src = st[32 * cg : 32 * cg + 1, :].rearrange(
                        "p (m j) -> p m j", m=8
                    )
                    nc.sync.dma_start(dstv[batch, :, cg, :], src)

            # ---------------- start/end logits (fp32, exact) ----------------
            for col0, outdram in ((3, slog), (9, elog)):
                pl = pq.tile([1, 128], f32, tag="psum")
                for hb in range(NH):
                    nc.tensor.matmul(
                        pl[:],
                        sm[:, col0 + hb : col0 + hb + 1],
                        xt(hb)[:, 0:LH],
                        start=(hb == 0),
                        stop=(hb == NH - 1),
                    )
                lt = sp.tile([1, 128], f32, tag="lt")
                nc.vector.tensor_copy(lt[:], pl[:])
                nc.sync.dma_start(outdram[:], lt[:])


    nc.compile()
    return nc


def _get_nc():
    if "nc" not in _built:
        _built["nc"] = _build()
    return _built["nc"]


def _make_in_maps(
    sequence_output, w_start, b_start, w_end, b_end, W1s, W1e, b1, W2, b2
):
    x = np.asarray(sequence_output, dtype=np.float32)

    def pack_w1(w):  # [H, H2] -> [128, NH*H2] partition-major bf16
        w = np.asarray(w, np.float32).reshape(NH, 128, H2).transpose(1, 0, 2)
        return np.ascontiguousarray(w.reshape(128, NH * H2)).astype(ml_dtypes.bfloat16)

    sml = np.zeros((128, 16), np.float32)
    sml[:, 0:3] = np.asarray(b1, np.float32).reshape(3, 128).T
    sml[:, 3:9] = np.asarray(w_start, np.float32).reshape(6, 128).T
    sml[:, 9:15] = np.asarray(w_end, np.float32).reshape(6, 128).T
    sml[:, 15] = float(np.asarray(b2))
    w2r = np.repeat(
        np.asarray(W2, np.float32).reshape(NK, 128, 1).transpose(1, 0, 2), 32, axis=2
    )  # [128, NK, 32]
    shared = {
        "W1e": pack_w1(W1e),
        "W1s": pack_w1(W1s),
        "W2r": np.ascontiguousarray(w2r.reshape(128, NK * 32)).astype(
            ml_dtypes.bfloat16
        ),
        "sml": sml,
    }
    in_maps = []
    for core in range(NCORES):
        b, ih = core // 2, core % 2
        xTr = np.roll(x[b].T, -ih * LH, axis=1)  # [H, L], own rows at cols 0..127
        xp = xTr.reshape(NH, 128, L).transpose(1, 0, 2)  # [128, NH, L]
        m = dict(shared)
        m["xT"] = np.ascontiguousarray(xp.reshape(128, NH * L))
        in_maps.append(m)
    return in_maps


def kernel(
    sequence_output, w_start, b_start, w_end, b_end, W1s, W1e, b1, W2, b2
):
    from concourse.bass_utils import run_bass_kernel_spmd

    nc = _get_nc()
    in_maps = _make_in_maps(
        sequence_output, w_start, b_start, w_end, b_end, W1s, W1e, b1, W2, b2
    )
    res = run_bass_kernel_spmd(nc, in_maps, core_ids=list(range(NCORES)))
    span = np.empty((B, L, L), np.float32)
    sl = np.empty((B, L), np.float32)
    el = np.empty((B, L), np.float32)
    bs = float(np.asarray(b_start))
    be = float(np.asarray(b_end))
    for core in range(NCORES):
        b, ih = core // 2, core % 2
        r = res.results[core]
        span[b, ih * LH : (ih + 1) * LH, :] = np.roll(r["span"], ih * LH, axis=1)
        sl[b, ih * LH : (ih + 1) * LH] = r["slog"] + bs
        el[b, ih * LH : (ih + 1) * LH] = r["elog"] + be
    return (sl, el, span)


# revision 18
# speedup vs baseline: 1.2060x; 1.0242x over previous
"""Trainium2 Bass kernel for BertForSpanAspectExtraction span scoring.

Computes, for x = sequence_output [B=4, L=256, H=768]:
  start_logits = x @ w_start + b_start                      [B, L]
  end_logits   = x @ w_end   + b_end                        [B, L]
  span_sig     = sigmoid(relu(s_i + e_j + b1) @ W2 + b2)    [B, L, L]
with s = x @ W1s, e = x @ W1e  (H2 = 384).

Sharding: 8 cores = (b, i-half).  Each core receives a column-rotated x[b].T
so its own 128 start-rows are always columns 0..127 (uniform SPMD program);
the host un-rotates the span columns after gathering.

Per core:
  - Inputs arrive host-pre-tiled partition-major so each tensor loads with one
    DMA of 128 large descriptors, spread over the SP/ACT HWDGE rings + SWDGE.
  - PE: bf16 projections e_T [384,256], s_T [384,128] (k on partitions),
    fp32 start/end logit rows.
  - Grid: per (i, kblock) one fused h = relu(e_T + s_col) op ([128,256] bf16,
    per-partition-scalar broadcast), split across ScalarE / VectorE / GpSimd;
    pairs of i's share one [128,512] rhs so each K=128 W2-reduction matmul
    streams N=512 into a full PSUM bank at col-group partition 32*(i%4).
  - Sigmoid evacuates two banks [128,1024] per ScalarE op (16 span rows).
"""

import sys

if "/opt/trn_rl_repo" not in sys.path:
    sys.path.insert(0, "/opt/trn_rl_repo")

import ml_dtypes
import numpy as np

B, L, H = 4, 256, 768
H2 = 384
NCORES = 8
LH = 128  # start-rows per core
NH = H // 128  # 6 contraction blocks
NK = H2 // 128  # 3 k blocks

# grid-op engine split (of 384 ops): ScalarE share, rest VectorE
ACT_OPS = 86
H_BUFS = 12

_built = {}


def _build():
    import concourse.bacc as bacc
    import concourse.mybir as mybir
    import concourse.tile as tile

    f32 = mybir.dt.float32
    bf16 = mybir.dt.bfloat16
    AF = mybir.ActivationFunctionType
    OP = mybir.AluOpType

    nc = bacc.Bacc("TRN2", debug=False, target_bir_lowering=False)

    # host-pre-tiled, partition-major inputs (one DMA each, 128 big descriptors)
    xT = nc.dram_tensor("xT", [128, NH * L], f32, kind="ExternalInput").ap()
    xTb = nc.dram_tensor("xTb", [128, NH * L], bf16, kind="ExternalInput").ap()
    W1e = nc.dram_tensor("W1e", [128, NH * H2], bf16, kind="ExternalInput").ap()
    W1s = nc.dram_tensor("W1s", [128, NH * H2], bf16, kind="ExternalInput").ap()
    W2r = nc.dram_tensor("W2r", [128, NK * 32], bf16, kind="ExternalInput").ap()
    # smalls cols: b1[kb]@kb (3) | w_start[hb]@3+hb (6) | w_end[hb]@9+hb (6) | b2@15
    sml = nc.dram_tensor("sml", [128, 16], f32, kind="ExternalInput").ap()

    span = nc.dram_tensor("span", [LH, L], f32, kind="ExternalOutput").ap()
    slog = nc.dram_tensor("slog", [LH], f32, kind="ExternalOutput").ap()
    elog = nc.dram_tensor("elog", [LH], f32, kind="ExternalOutput").ap()

    with tile.TileContext(nc) as tc:
        with (
            tc.tile_pool(name="persist", bufs=1) as pp,
            tc.tile_pool(name="hpool", bufs=H_BUFS) as hp,
            tc.tile_pool(name="psum", bufs=2, space="PSUM") as pq,
            tc.tile_pool(name="stage", bufs=2) as sp,
        ):
            # ---------------- input loads (3 rings in parallel) ----------------
            xtball = pp.tile([128, NH * L], bf16, tag="xtball", name="xtball")
            nc.sync.dma_start(xtball[:], xTb[:])
            w1e = pp.tile([128, NH * H2], bf16, tag="w1e", name="w1e")
            nc.scalar.dma_start(w1e[:], W1e[:])
            w1s = pp.tile([128, NH * H2], bf16, tag="w1s", name="w1s")
            nc.gpsimd.dma_start(w1s[:], W1s[:])
            w2a = pp.tile([128, NK * 32], bf16, tag="w2a", name="w2a")
            nc.gpsimd.dma_start(w2a[:], W2r[:])
            sm = pp.tile([128, 16], f32, tag="sm", name="sm")
            nc.gpsimd.dma_start(sm[:], sml[:])
            # fp32 x only feeds the end-of-kernel logit matmuls; load it last
            xta = pp.tile([128, NH * L], f32, tag="xta", name="xta")
            nc.sync.dma_start(xta[:], xT[:])

            def xt(hb):
                return xta[:, hb * L : (hb + 1) * L]

            def w1(t, hb, kb):
                return t[:, hb * H2 + kb * 128 : hb * H2 + (kb + 1) * 128]

            xtb = [xtball[:, hb * L : (hb + 1) * L] for hb in range(NH)]

            # ---------------- projections ----------------
            eT = [
                pp.tile([128, L], bf16, tag=f"eT{kb}", name=f"eT{kb}")
                for kb in range(NK)
            ]
            sT = [
                pp.tile([128, LH], f32, tag=f"sT{kb}", name=f"sT{kb}")
                for kb in range(NK)
            ]
            for kb in range(NK):
                pe = pq.tile([128, 2048], f32, tag="psum")
                for hb in range(NH):
                    nc.tensor.matmul(
                        pe[:, 0:L],
                        w1(w1e, hb, kb),
                        xtb[hb],
                        start=(hb == 0),
                        stop=(hb == NH - 1),
                    )
                nc.scalar.activation(eT[kb][:], pe[:, 0:L], AF.Identity, bias=0.0)
            for kb in range(NK):
                ps = pq.tile([128, 2048], f32, tag="psum")
                for hb in range(NH):
                    nc.tensor.matmul(
                        ps[:, 0:LH],
                        w1(w1s, hb, kb),
                        xtb[hb][0:128, 0:LH],
                        start=(hb == 0),
                        stop=(hb == NH - 1),
                    )
                # fold b1 into the mandatory PSUM->SBUF evacuation
                nc.scalar.activation(
                    sT[kb][:], ps[:, 0:LH], AF.Identity, bias=sm[:, kb : kb + 1]
                )

            # ---------------- span grid ----------------
            # local row i = 32*batch + m*4 + cg at PSUM partition 32*cg, free
            # m*256; i-pairs (m=2mp, 2mp+1) share one [128,512] bf16 rhs and
            # one full-bank accumulation group per (cg, bank=mp)
            opct = 0
            n_grid_ops = LH * NK
            dstv = span.rearrange("(bt m cg) j -> bt m cg j", bt=LH // 32, m=8, cg=4)
            for batch in range(LH // 32):
                pt = pq.tile([128, 2048], f32, tag="psum")
                st = sp.tile([128, 2048], f32, tag="stage")
                for mp in range(4):
                    for kb in range(NK):
                        for cg in range(4):
                            h2 = hp.tile([128, 512], bf16, tag="h")
                            for half in range(2):
                                i = batch * 32 + (2 * mp + half) * 4 + cg
                                dst = h2[:, half * 256 : (half + 1) * 256]
                                t = opct
                                opct += 1
                                if (t * ACT_OPS) % n_grid_ops < ACT_OPS:
                                    nc.scalar.activation(
                                        dst,
                                        eT[kb][:],
                                        AF.Relu,
                                        bias=sT[kb][:, i : i + 1],
                                    )
                                else:
                                    nc.vector.tensor_scalar(
                                        dst,
                                        eT[kb][:],
                                        sT[kb][:, i : i + 1],
                                        0.0,
                                        op0=OP.add,
                                        op1=OP.max,
                                    )
                            nc.tensor.matmul(
                                pt[32 * cg : 32 * cg + 32, mp * 512 : (mp + 1) * 512],
                                w2a[:, kb * 32 : (kb + 1) * 32],
                                h2[:],
                                start=(kb == 0),
                                stop=(kb == NK - 1),
                                tile_position=(0, 32 * cg),
                                # sim's group-check mis-maps partition-offset
                                # outputs; functional pending-zero model is fine
                                skip_group_check=True,
                            )
                for half in range(2):
                    nc.scalar.activation(
                        st[:, half * 1024 : (half + 1) * 1024],
                        pt[:, half * 1024 : (half + 1) * 1024],
                        AF.Sigmoid,
                        bias=sm[:, 15:16],
                    )
                for cg in range(4):
                    src = st[32 * cg : 32 * cg + 1, :].rearrange(
                        "p (m j) -> p m j", m=8
                    )
                    nc.sync.dma_start(dstv[batch, :, cg, :], src)

            # ---------------- start/end logits (fp32, exact) ----------------
            for col0, outdram in ((3, slog), (9, elog)):
                pl = pq.tile([1, 128], f32, tag="psum")
                for hb in range(NH):
                    nc.tensor.matmul(
                        pl[:],
                        sm[:, col0 + hb : col0 + hb + 1],
                        xt(hb)[:, 0:LH],
                        start=(hb == 0),
                        stop=(hb == NH - 1),
                    )
                lt = sp.tile([1, 128], f32, tag="lt")
                nc.vector.tensor_copy(lt[:], pl[:])
                nc.sync.dma_start(outdram[:], lt[:])


    nc.compile()
    return nc


def _get_nc():
    if "nc" not in _built:
        _built["nc"] = _build()
    return _built["nc"]


def _make_in_maps(
    sequence_output, w_start, b_start, w_end, b_end, W1s, W1e, b1, W2, b2
):
    x = np.asarray(sequence_output, dtype=np.float32)

    def pack_w1(w):  # [H, H2] -> [128, NH*H2] partition-major bf16
        w = np.asarray(w, np.float32).reshape(NH, 128, H2).transpose(1, 0, 2)
        return np.ascontiguousarray(w.reshape(128, NH * H2)).astype(ml_dtypes.bfloat16)

    sml = np.zeros((128, 16), np.float32)
    sml[:, 0:3] = np.asarray(b1, np.float32).reshape(3, 128).T
    sml[:, 3:9] = np.asarray(w_start, np.float32).reshape(6, 128).T
    sml[:, 9:15] = np.asarray(w_end, np.float32).reshape(6, 128).T
    sml[:, 15] = float(np.asarray(b2))
    w2r = np.repeat(
        np.asarray(W2, np.float32).reshape(NK, 128, 1).transpose(1, 0, 2), 32, axis=2
    )  # [128, NK, 32]
    shared = {
        "W1e": pack_w1(W1e),
        "W1s": pack_w1(W1s),
        "W2r": np.ascontiguousarray(w2r.reshape(128, NK * 32)).astype(
            ml_dtypes.bfloat16
        ),
        "sml": sml,
    }
    in_maps = []
    for core in range(NCORES):
        b, ih = core // 2, core % 2
        xTr = np.roll(x[b].T, -ih * LH, axis=1)  # [H, L], own rows at cols 0..127
        xp = xTr.reshape(NH, 128, L).transpose(1, 0, 2)  # [128, NH, L]
        m = dict(shared)
        xflat = np.ascontiguousarray(xp.reshape(128, NH * L))
        m["xT"] = xflat
        m["xTb"] = xflat.astype(ml_dtypes.bfloat16)
        in_maps.append(m)
    return in_maps


def kernel(
    sequence_output, w_start, b_start, w_end, b_end, W1s, W1e, b1, W2, b2
):
    from concourse.bass_utils import run_bass_kernel_spmd

    nc = _get_nc()
    in_maps = _make_in_maps(
        sequence_output, w_start, b_start, w_end, b_end, W1s, W1e, b1, W2, b2
    )
    res = run_bass_kernel_spmd(nc, in_maps, core_ids=list(range(NCORES)))
    span = np.empty((B, L, L), np.float32)
    sl = np.empty((B, L), np.float32)
    el = np.empty((B, L), np.float32)
    bs = float(np.asarray(b_start))
    be = float(np.asarray(b_end))
    for core in range(NCORES):
        b, ih = core // 2, core % 2
        r = res.results[core]
        span[b, ih * LH : (ih + 1) * LH, :] = np.roll(r["span"], ih * LH, axis=1)
        sl[b, ih * LH : (ih + 1) * LH] = r["slog"] + bs
        el[b, ih * LH : (ih + 1) * LH] = r["elog"] + be
    return (sl, el, span)
